# revision 12
# baseline (speedup 1.0000x reference)
"""Trainium2 Bass kernel for nn_AttentionModule (B=8, C=256, L=2048, D=32).

Per-batch computation (data-parallel: one batch per NeuronCore, 8 cores):
    qT = Wq @ x + bq            # (D, L)
    kT = Wk @ x + bk            # (D, L)
    vT = x.T @ (g*Wv).T         # (L, C)   -- gamma folded into Wv host-side
    ST = kT.T @ qT              # (L_j, L_i) = S[i,j] transposed
    E  = exp(ST)                # no max-subtraction: max|S| ~ 46, exp fits
    Z  = sum_j E[j, i]
    y  = U * (1/Z) + x'         # U = vT.T @ E;  x' = x + g*bv (host-folded)

Restructured v2: i processed in 2 halves of 1024 columns; inside a half the
16 j-blocks stream through a 3-stage pipeline (S matmul -> exp -> {U matmul,
Z accumulate}) with wide [128,1024] tiles so per-instruction overheads
amortize.  S uses 32-row PE packing with the row-group pair alternating
per j-block so next block's LDWEIGHTS overlaps in-flight matmuls.  Z is
partition-reduced with an all-ones 128x128 stationary, which lands the
row sums already broadcast across all partitions (no transpose / no
broadcast step); 1/Z via reciprocal_approx_fast.  gamma and the v/q-k
biases are folded into the weights / residual host-side.
"""

import numpy as np

B, C, L, D = 8, 256, 2048, 32
H = 1024  # i-half width
NCORES = 8

_cache = {}


def _build_nc():
    from contextlib import ExitStack

    import concourse.bacc as bacc
    import concourse.tile as tile
    from concourse import mybir

    f32 = mybir.dt.float32
    bf16 = mybir.dt.bfloat16
    EXP = mybir.ActivationFunctionType.Exp
    IDENT = mybir.ActivationFunctionType.Identity

    nc = bacc.Bacc("TRN2", target_bir_lowering=False, debug=False)

    x_d = nc.dram_tensor("x", [C, L], bf16, kind="ExternalInput")
    xbq_d = [nc.dram_tensor(f"xb{q}", [C, 512], bf16, kind="ExternalInput")
             for q in range(4)]
    wqk_d = nc.dram_tensor("wqk", [C, 2 * D], bf16, kind="ExternalInput")
    wvT_d = nc.dram_tensor("wvT", [C, C], bf16, kind="ExternalInput")
    bqk_d = nc.dram_tensor("bqk", [2 * D, 1], f32, kind="ExternalInput")
    ones_d = nc.dram_tensor("ones", [128, 128], bf16, kind="ExternalInput")
    y_d = nc.dram_tensor("y", [C, L], f32, kind="ExternalOutput")

    x_ap = x_d.ap()
    y_ap = y_d.ap()

    with tile.TileContext(nc) as tc, ExitStack() as ctx:
        singles = ctx.enter_context(tc.tile_pool(name="singles", bufs=1))
        big = ctx.enter_context(tc.tile_pool(name="big", bufs=1))
        ps = ctx.enter_context(tc.tile_pool(name="ps", bufs=2, space="PSUM"))
        up = ctx.enter_context(tc.tile_pool(name="up", bufs=1, space="PSUM"))
        epool = ctx.enter_context(tc.tile_pool(name="epool", bufs=4))
        ypool = ctx.enter_context(tc.tile_pool(name="ypool", bufs=4))
        zpool = ctx.enter_context(tc.tile_pool(name="zpool", bufs=1))
        rpool = ctx.enter_context(tc.tile_pool(name="rpool", bufs=2))

        # ---- load compute inputs (weights first, then xb; bf16 residual last) ----
        wqk_sb, wvT_sb = [], []
        for ct in range(2):
            tq = singles.tile([128, 2 * D], bf16, tag=f"wqk{ct}")
            nc.sync.dma_start(out=tq[:], in_=wqk_d.ap()[ct * 128:(ct + 1) * 128, :])
            wqk_sb.append(tq)
            tv = singles.tile([128, C], bf16, tag=f"wv{ct}")
            nc.sync.dma_start(out=tv[:], in_=wvT_d.ap()[ct * 128:(ct + 1) * 128, :])
            wvT_sb.append(tv)
        bqk_sb = singles.tile([2 * D, 1], f32, tag="bqk")
        nc.sync.dma_start(out=bqk_sb[:], in_=bqk_d.ap()[:, :])
        ones_sb = singles.tile([128, 128], bf16, tag="ones")
        nc.sync.dma_start(out=ones_sb[:], in_=ones_d.ap()[:, :])
        xb_sb = []
        for ct in range(2):
            tb = big.tile([128, L], bf16, tag=f"xb{ct}")
            xb_sb.append(tb)
        for q in range(4):
            for ct in range(2):
                nc.sync.dma_start(
                    out=xb_sb[ct][:, q * 512:(q + 1) * 512],
                    in_=xbq_d[q].ap()[ct * 128:(ct + 1) * 128, :],
                )

        # ---- PE warm-up: tiny matmuls keep the HAM clock gate open while
        # input DMAs land, so the projections run at 2.4 GHz not 1.2.
        warmsrc = singles.tile([128, 2], bf16, tag="warmsrc")
        warm_e = singles.tile([1, 2], bf16, tag="warm_e")
        nc.gpsimd.memset(warmsrc[:], 1.0)
        warmps = ps.tile([1, 2], f32, tag="stp", name="warmps")
        # hoist the exp ACT_TABLE_LOAD off the critical path
        nc.scalar.activation(warm_e[:], warmsrc[0:1, :], EXP)

        def warm_burst(src, n):
            for _ in range(n):
                nc.tensor.matmul(
                    warmps[0:1, 0:1], lhsT=src, rhs=src,
                    start=True, stop=True,
                )
        warm_burst(warmsrc[:, 0:1], 40)
        warm_burst(wqk_sb[0][:, 0:1], 20)
        warm_burst(wvT_sb[0][:, 0:1], 20)

        # ---- q/k projection: psum rows 0:32 = q, 32:64 = k ----
        qk64 = big.tile([2 * D, L], bf16, tag="qk64")
        qT4 = big.tile([128, L], bf16, tag="qT4")
        kT4r = big.tile([128, L], bf16, tag="kT4r")
        for p in range(2):
            qkp = up.tile([2 * D, H], f32, tag=f"u{p}")
            for hi in range(2):
                it = 2 * p + hi
                for ct in range(2):
                    nc.tensor.matmul(
                        qkp[0:2 * D, hi * 512:(hi + 1) * 512],
                        lhsT=wqk_sb[ct][:],
                        rhs=xb_sb[ct][:, it * 512:(it + 1) * 512],
                        start=(ct == 0),
                        stop=(ct == 1),
                    )
            nc.scalar.activation(
                qk64[0:2 * D, p * H:(p + 1) * H], qkp[0:2 * D, :], IDENT,
                bias=bqk_sb[:],
            )
            # replicate this half of q and k across all four 32-row strips
            for g in range(4):
                nc.sync.dma_start(
                    out=qT4[32 * g:32 * (g + 1), p * H:(p + 1) * H],
                    in_=qk64[0:D, p * H:(p + 1) * H],
                )
                nc.sync.dma_start(
                    out=kT4r[32 * g:32 * (g + 1), p * H:(p + 1) * H],
                    in_=qk64[D:2 * D, p * H:(p + 1) * H],
                )

        # ---- v projection + attention ----
        vT_sb = big.tile([128, 16 * C], bf16, tag="vT")

        def emit_vgrp(grp):
            vp = up.tile([128, H], f32, tag=f"u{grp % 2}", name="vp")
            for q4 in range(4):
                lb = 4 * grp + q4
                for ct in range(2):
                    nc.tensor.matmul(
                        vp[:, q4 * C:(q4 + 1) * C],
                        lhsT=xb_sb[ct][:, lb * 128:(lb + 1) * 128],
                        rhs=wvT_sb[ct][:],
                        start=(ct == 0),
                        stop=(ct == 1),
                    )
            nc.vector.tensor_copy(vT_sb[:, grp * H:(grp + 1) * H], vp[:, :])

        # S for block jb+1 is emitted BEFORE U for block jb so the in-order
        # PE queue computes next scores while ACT runs exp on the current
        # block (software pipelining).
        def emit_S(h, jb):
            gp = 2 * (jb % 2)  # alternate row-group pairs {0,1} / {2,3}
            stp = ps.tile([128, H], f32, tag="stp", name="stp")
            for c2 in range(2):
                g = gp + c2
                nc.tensor.matmul(
                    stp[:, c2 * 512:(c2 + 1) * 512],
                    lhsT=kT4r[32 * g:32 * (g + 1), jb * 128:(jb + 1) * 128],
                    rhs=qT4[32 * g:32 * (g + 1), h * H + c2 * 512:h * H + (c2 + 1) * 512],
                    start=True,
                    stop=True,
                    tile_position=(32 * g, 0),
                )
            return stp

        emit_vgrp(0)
        stp_q = [emit_S(0, 0)]
        emit_vgrp(1)
        stp_q.append(emit_S(0, 1))
        emit_vgrp(2)
        emit_vgrp(3)

        # fp32 residual x' = x + g*bv (folded host-side)
        x_sb = []
        for ct in range(2):
            t = big.tile([128, L], bf16, tag=f"x{ct}")
            nc.sync.dma_start(out=t[:], in_=x_ap[ct * 128:(ct + 1) * 128, :])
            x_sb.append(t)

        # ---- attention, two i-halves of 1024 ----
        for h in range(2):
            zA = zpool.tile([128, H], bf16, tag="zA")
            zB = zpool.tile([128, H], bf16, tag="zB")
            u_t = [up.tile([128, H], f32, tag=f"u{ct}", name=f"u{ct}") for ct in range(2)]
            zp = None
            for jb in range(16):
                stp = stp_q.pop(0)
                e = epool.tile([128, H], bf16, tag="e")
                nc.scalar.activation(e[:], stp[:], EXP)
                nxt = jb + 2
                if nxt < 16:
                    stp_q.append(emit_S(h, nxt))
                elif h == 0:
                    stp_q.append(emit_S(1, nxt - 16))

                def emit_U():
                    for ct in range(2):
                        for c2 in range(2):
                            nc.tensor.matmul(
                                u_t[ct][:, c2 * 512:(c2 + 1) * 512],
                                lhsT=vT_sb[:, jb * C + ct * 128:jb * C + (ct + 1) * 128],
                                rhs=e[:, c2 * 512:(c2 + 1) * 512],
                                start=(jb == 0),
                                stop=(jb == 15),
                            )

                def emit_z15():
                    for c2 in range(2):
                        sl = slice(c2 * 512, (c2 + 1) * 512)
                        nc.tensor.matmul(zp[:, sl], lhsT=ones_sb[:], rhs=e[:, sl],
                                         start=False, stop=True)

                # Z accumulation: blocks 0-13 into two parity accumulators on
                # DVE; blocks 14/15 go straight into the Z-reduce matmul group
                # so the tail doesn't wait on the last two DVE adds.  On the
                # final half the block-15 Z matmuls go ahead of U so the
                # reciprocal overlaps the last U matmuls.
                if jb < 2:
                    zt = zA if jb % 2 == 0 else zB
                    emit_U()
                    nc.vector.tensor_copy(zt[:], e[:])
                elif jb < 14:
                    zt = zA if jb % 2 == 0 else zB
                    emit_U()
                    nc.vector.tensor_add(zt[:], zt[:], e[:])
                elif jb == 14:
                    emit_U()
                    # Z partition-reduce; ones stationary broadcasts the sums
                    zp = ps.tile([128, H], f32, tag="stp", name="zp")
                    for c2 in range(2):
                        sl = slice(c2 * 512, (c2 + 1) * 512)
                        nc.tensor.matmul(zp[:, sl], lhsT=ones_sb[:], rhs=zA[:, sl],
                                         start=True, stop=False)
                        nc.tensor.matmul(zp[:, sl], lhsT=ones_sb[:], rhs=zB[:, sl],
                                         start=False, stop=False)
                        nc.tensor.matmul(zp[:, sl], lhsT=ones_sb[:], rhs=e[:, sl],
                                         start=False, stop=False)
                elif h == 1:
                    emit_z15()
                    emit_U()
                else:
                    emit_U()
                    emit_z15()
            rb = rpool.tile([128, H], f32, tag="rb")
            nc.vector.reciprocal_approx_fast(out=rb[:], in_=zp[:])
            for ct in range(2):
                yt = ypool.tile([128, H], f32, tag="y")
                nc.vector.tensor_mul(yt[:], u_t[ct][:], rb[:])
                eng = nc.gpsimd if ct == 0 else nc.vector
                eng.tensor_add(yt[:], yt[:], x_sb[ct][:, h * H:(h + 1) * H])
                nc.sync.dma_start(
                    out=y_ap[ct * 128:(ct + 1) * 128, h * H:(h + 1) * H], in_=yt[:]
                )

    nc.compile()
    return nc


def get_nc():
    if "nc" not in _cache:
        _cache["nc"] = _build_nc()
    return _cache["nc"]


def make_in_maps(x, Wq, bq, Wk, bk, Wv, bv, gamma):
    import ml_dtypes

    bf = ml_dtypes.bfloat16
    x = np.asarray(x, dtype=np.float32)
    g = float(np.asarray(gamma, np.float32).reshape(-1)[0])
    shared = {
        "wqk": np.ascontiguousarray(
            np.concatenate([np.asarray(Wq, np.float32).T,
                            np.asarray(Wk, np.float32).T], axis=1)).astype(bf),
        "wvT": np.ascontiguousarray((g * np.asarray(Wv, np.float32)).T).astype(bf),
        "bqk": np.concatenate([np.asarray(bq, np.float32).reshape(D, 1),
                               np.asarray(bk, np.float32).reshape(D, 1)], axis=0),
        "ones": np.ones((128, 128), bf),
    }
    xres = x + g * np.asarray(bv, np.float32)[None, :, None]  # (B, C, L)
    maps = []
    for b in range(B):
        m = dict(shared, x=np.ascontiguousarray(xres[b]).astype(bf))
        xbb = np.asarray(x[b]).astype(bf)
        for q in range(4):
            m[f"xb{q}"] = np.ascontiguousarray(xbb[:, q * 512:(q + 1) * 512])
        maps.append(m)
    return maps


def kernel(x, Wq, bq, Wk, bk, Wv, bv, gamma):
    from concourse.bass_utils import run_bass_kernel_spmd

    nc = get_nc()
    in_maps = make_in_maps(x, Wq, bq, Wk, bk, Wv, bv, gamma)
    res = run_bass_kernel_spmd(nc, in_maps, list(range(NCORES)))
    return np.stack([res.results[b]["y"] for b in range(B)], axis=0)


# revision 14
# speedup vs baseline: 1.0008x; 1.0008x over previous
"""Trainium2 Bass kernel for nn_AttentionModule (B=8, C=256, L=2048, D=32).

Per-batch computation (data-parallel: one batch per NeuronCore, 8 cores):
    qT = Wq @ x + bq            # (D, L)
    kT = Wk @ x + bk            # (D, L)
    vT = x.T @ (g*Wv).T         # (L, C)   -- gamma folded into Wv host-side
    ST = kT.T @ qT              # (L_j, L_i) = S[i,j] transposed
    E  = exp(ST)                # no max-subtraction: max|S| ~ 46, exp fits
    Z  = sum_j E[j, i]
    y  = U * (1/Z) + x'         # U = vT.T @ E;  x' = x + g*bv (host-folded)

Restructured v2: i processed in 2 halves of 1024 columns; inside a half the
16 j-blocks stream through a 3-stage pipeline (S matmul -> exp -> {U matmul,
Z accumulate}) with wide [128,1024] tiles so per-instruction overheads
amortize.  S uses 32-row PE packing with the row-group pair alternating
per j-block so next block's LDWEIGHTS overlaps in-flight matmuls.  Z is
partition-reduced with an all-ones 128x128 stationary, which lands the
row sums already broadcast across all partitions (no transpose / no
broadcast step); 1/Z via reciprocal_approx_fast.  gamma and the v/q-k
biases are folded into the weights / residual host-side.
"""

import numpy as np

B, C, L, D = 8, 256, 2048, 32
H = 1024  # i-half width
NCORES = 8

_cache = {}


def _build_nc():
    from contextlib import ExitStack

    import concourse.bacc as bacc
    import concourse.tile as tile
    from concourse import mybir

    f32 = mybir.dt.float32
    bf16 = mybir.dt.bfloat16
    EXP = mybir.ActivationFunctionType.Exp
    IDENT = mybir.ActivationFunctionType.Identity

    nc = bacc.Bacc("TRN2", target_bir_lowering=False, debug=False)

    x_d = nc.dram_tensor("x", [C, L], bf16, kind="ExternalInput")
    xbq_d = [nc.dram_tensor(f"xb{q}", [C, 512], bf16, kind="ExternalInput")
             for q in range(4)]
    wqka_d = nc.dram_tensor("wqka", [C, 128], bf16, kind="ExternalInput")
    wqkb_d = nc.dram_tensor("wqkb", [C, 128], bf16, kind="ExternalInput")
    wvT_d = nc.dram_tensor("wvT", [C, C], bf16, kind="ExternalInput")
    bqka_d = nc.dram_tensor("bqka", [128, 1], f32, kind="ExternalInput")
    bqkb_d = nc.dram_tensor("bqkb", [128, 1], f32, kind="ExternalInput")
    ones_d = nc.dram_tensor("ones", [128, 128], bf16, kind="ExternalInput")
    y_d = nc.dram_tensor("y", [C, L], f32, kind="ExternalOutput")

    x_ap = x_d.ap()
    y_ap = y_d.ap()

    with tile.TileContext(nc) as tc, ExitStack() as ctx:
        singles = ctx.enter_context(tc.tile_pool(name="singles", bufs=1))
        big = ctx.enter_context(tc.tile_pool(name="big", bufs=1))
        ps = ctx.enter_context(tc.tile_pool(name="ps", bufs=2, space="PSUM"))
        up = ctx.enter_context(tc.tile_pool(name="up", bufs=1, space="PSUM"))
        epool = ctx.enter_context(tc.tile_pool(name="epool", bufs=4))
        ypool = ctx.enter_context(tc.tile_pool(name="ypool", bufs=4))
        zpool = ctx.enter_context(tc.tile_pool(name="zpool", bufs=1))
        rpool = ctx.enter_context(tc.tile_pool(name="rpool", bufs=2))

        # ---- load compute inputs (weights first, then xb; bf16 residual last) ----
        wqka_sb, wqkb_sb, wvT_sb = [], [], []
        for ct in range(2):
            ta = singles.tile([128, 128], bf16, tag=f"wqka{ct}")
            nc.sync.dma_start(out=ta[:], in_=wqka_d.ap()[ct * 128:(ct + 1) * 128, :])
            wqka_sb.append(ta)
            tb2 = singles.tile([128, 128], bf16, tag=f"wqkb{ct}")
            nc.sync.dma_start(out=tb2[:], in_=wqkb_d.ap()[ct * 128:(ct + 1) * 128, :])
            wqkb_sb.append(tb2)
            tv = singles.tile([128, C], bf16, tag=f"wv{ct}")
            nc.sync.dma_start(out=tv[:], in_=wvT_d.ap()[ct * 128:(ct + 1) * 128, :])
            wvT_sb.append(tv)
        bqka_sb = singles.tile([128, 1], f32, tag="bqka")
        nc.sync.dma_start(out=bqka_sb[:], in_=bqka_d.ap()[:, :])
        bqkb_sb = singles.tile([128, 1], f32, tag="bqkb")
        nc.sync.dma_start(out=bqkb_sb[:], in_=bqkb_d.ap()[:, :])
        ones_sb = singles.tile([128, 128], bf16, tag="ones")
        nc.sync.dma_start(out=ones_sb[:], in_=ones_d.ap()[:, :])
        xb_sb = []
        for ct in range(2):
            tb = big.tile([128, L], bf16, tag=f"xb{ct}")
            xb_sb.append(tb)
        for q in range(4):
            for ct in range(2):
                nc.sync.dma_start(
                    out=xb_sb[ct][:, q * 512:(q + 1) * 512],
                    in_=xbq_d[q].ap()[ct * 128:(ct + 1) * 128, :],
                )

        # ---- PE warm-up: tiny matmuls keep the HAM clock gate open while
        # input DMAs land, so the projections run at 2.4 GHz not 1.2.
        warmsrc = singles.tile([128, 2], bf16, tag="warmsrc")
        warm_e = singles.tile([1, 2], bf16, tag="warm_e")
        nc.gpsimd.memset(warmsrc[:], 1.0)
        warmps = ps.tile([1, 2], f32, tag="stp", name="warmps")
        # hoist the exp ACT_TABLE_LOAD off the critical path
        nc.scalar.activation(warm_e[:], warmsrc[0:1, :], EXP)

        def warm_burst(src, n):
            for _ in range(n):
                nc.tensor.matmul(
                    warmps[0:1, 0:1], lhsT=src, rhs=src,
                    start=True, stop=True,
                )
        warm_burst(warmsrc[:, 0:1], 40)
        warm_burst(wqka_sb[0][:, 0:1], 20)
        warm_burst(wvT_sb[0][:, 0:1], 20)

        # ---- q/k projection, strip-interleaved ----
        # proja strips = [q|k|q|k], projb strips = [k|q|k|q]: every 32-row
        # strip g has q in one tile and k in the other, so the packed score
        # matmuls need no replication DMAs at all.
        proja = big.tile([128, L], bf16, tag="proja")
        projb = big.tile([128, L], bf16, tag="projb")
        for p in range(2):
            for tile_w, tile_b, dst in ((wqka_sb, bqka_sb, proja),
                                        (wqkb_sb, bqkb_sb, projb)):
                qkp = up.tile([128, H], f32, tag=f"u{0 if dst is proja else 1}",
                              name="qkp")
                for hi in range(2):
                    it = 2 * p + hi
                    for ct in range(2):
                        nc.tensor.matmul(
                            qkp[:, hi * 512:(hi + 1) * 512],
                            lhsT=tile_w[ct][:],
                            rhs=xb_sb[ct][:, it * 512:(it + 1) * 512],
                            start=(ct == 0),
                            stop=(ct == 1),
                        )
                nc.scalar.activation(
                    dst[:, p * H:(p + 1) * H], qkp[:, :], IDENT, bias=tile_b[:],
                )

        # ---- v projection + attention ----
        vT_sb = big.tile([128, 16 * C], bf16, tag="vT")

        def emit_vgrp(grp):
            vp = up.tile([128, H], f32, tag=f"u{grp % 2}", name="vp")
            for q4 in range(4):
                lb = 4 * grp + q4
                for ct in range(2):
                    nc.tensor.matmul(
                        vp[:, q4 * C:(q4 + 1) * C],
                        lhsT=xb_sb[ct][:, lb * 128:(lb + 1) * 128],
                        rhs=wvT_sb[ct][:],
                        start=(ct == 0),
                        stop=(ct == 1),
                    )
            nc.vector.tensor_copy(vT_sb[:, grp * H:(grp + 1) * H], vp[:, :])

        # S for block jb+1 is emitted BEFORE U for block jb so the in-order
        # PE queue computes next scores while ACT runs exp on the current
        # block (software pipelining).
        def emit_S(h, jb):
            gp = 2 * (jb % 2)  # alternate row-group pairs {0,1} / {2,3}
            stp = ps.tile([128, H], f32, tag="stp", name="stp")
            for c2 in range(2):
                g = gp + c2
                ktile = projb if g % 2 == 0 else proja
                qtile = proja if g % 2 == 0 else projb
                nc.tensor.matmul(
                    stp[:, c2 * 512:(c2 + 1) * 512],
                    lhsT=ktile[32 * g:32 * (g + 1), jb * 128:(jb + 1) * 128],
                    rhs=qtile[32 * g:32 * (g + 1), h * H + c2 * 512:h * H + (c2 + 1) * 512],
                    start=True,
                    stop=True,
                    tile_position=(32 * g, 0),
                )
            return stp

        emit_vgrp(0)
        stp_q = [emit_S(0, 0)]
        emit_vgrp(1)
        stp_q.append(emit_S(0, 1))
        emit_vgrp(2)
        emit_vgrp(3)

        # fp32 residual x' = x + g*bv (folded host-side)
        x_sb = []
        for ct in range(2):
            t = big.tile([128, L], bf16, tag=f"x{ct}")
            nc.sync.dma_start(out=t[:], in_=x_ap[ct * 128:(ct + 1) * 128, :])
            x_sb.append(t)

        # ---- attention, two i-halves of 1024 ----
        for h in range(2):
            zA = zpool.tile([128, H], bf16, tag="zA")
            zB = zpool.tile([128, H], bf16, tag="zB")
            u_t = [up.tile([128, H], f32, tag=f"u{ct}", name=f"u{ct}") for ct in range(2)]
            zp = None
            for jb in range(16):
                stp = stp_q.pop(0)
                e = epool.tile([128, H], bf16, tag="e")
                nc.scalar.activation(e[:], stp[:], EXP)
                nxt = jb + 2
                if nxt < 16:
                    stp_q.append(emit_S(h, nxt))
                elif h == 0:
                    stp_q.append(emit_S(1, nxt - 16))

                def emit_U():
                    for ct in range(2):
                        for c2 in range(2):
                            nc.tensor.matmul(
                                u_t[ct][:, c2 * 512:(c2 + 1) * 512],
                                lhsT=vT_sb[:, jb * C + ct * 128:jb * C + (ct + 1) * 128],
                                rhs=e[:, c2 * 512:(c2 + 1) * 512],
                                start=(jb == 0),
                                stop=(jb == 15),
                            )

                def emit_z15():
                    for c2 in range(2):
                        sl = slice(c2 * 512, (c2 + 1) * 512)
                        nc.tensor.matmul(zp[:, sl], lhsT=ones_sb[:], rhs=e[:, sl],
                                         start=False, stop=True)

                # Z accumulation: blocks 0-13 into two parity accumulators on
                # DVE; blocks 14/15 go straight into the Z-reduce matmul group
                # so the tail doesn't wait on the last two DVE adds.  On the
                # final half the block-15 Z matmuls go ahead of U so the
                # reciprocal overlaps the last U matmuls.
                if jb < 2:
                    zt = zA if jb % 2 == 0 else zB
                    emit_U()
                    nc.vector.tensor_copy(zt[:], e[:])
                elif jb < 14:
                    zt = zA if jb % 2 == 0 else zB
                    emit_U()
                    nc.vector.tensor_add(zt[:], zt[:], e[:])
                elif jb == 14:
                    emit_U()
                    # Z partition-reduce; ones stationary broadcasts the sums
                    zp = ps.tile([128, H], f32, tag="stp", name="zp")
                    for c2 in range(2):
                        sl = slice(c2 * 512, (c2 + 1) * 512)
                        nc.tensor.matmul(zp[:, sl], lhsT=ones_sb[:], rhs=zA[:, sl],
                                         start=True, stop=False)
                        nc.tensor.matmul(zp[:, sl], lhsT=ones_sb[:], rhs=zB[:, sl],
                                         start=False, stop=False)
                        nc.tensor.matmul(zp[:, sl], lhsT=ones_sb[:], rhs=e[:, sl],
                                         start=False, stop=False)
                elif h == 1:
                    emit_z15()
                    emit_U()
                else:
                    emit_U()
                    emit_z15()
            rb = rpool.tile([128, H], f32, tag="rb")
            nc.vector.reciprocal_approx_fast(out=rb[:], in_=zp[:])
            for ct in range(2):
                yt = ypool.tile([128, H], f32, tag="y")
                nc.vector.tensor_mul(yt[:], u_t[ct][:], rb[:])
                eng = nc.gpsimd if ct == 0 else nc.vector
                eng.tensor_add(yt[:], yt[:], x_sb[ct][:, h * H:(h + 1) * H])
                nc.sync.dma_start(
                    out=y_ap[ct * 128:(ct + 1) * 128, h * H:(h + 1) * H], in_=yt[:]
                )

    nc.compile()
    return nc


def get_nc():
    if "nc" not in _cache:
        _cache["nc"] = _build_nc()
    return _cache["nc"]


def make_in_maps(x, Wq, bq, Wk, bk, Wv, bv, gamma):
    import ml_dtypes

    bf = ml_dtypes.bfloat16
    x = np.asarray(x, dtype=np.float32)
    g = float(np.asarray(gamma, np.float32).reshape(-1)[0])
    WqT = np.asarray(Wq, np.float32).T
    WkT = np.asarray(Wk, np.float32).T
    bq_ = np.asarray(bq, np.float32).reshape(D, 1)
    bk_ = np.asarray(bk, np.float32).reshape(D, 1)
    shared = {
        "wqka": np.ascontiguousarray(
            np.concatenate([WqT, WkT, WqT, WkT], axis=1)).astype(bf),
        "wqkb": np.ascontiguousarray(
            np.concatenate([WkT, WqT, WkT, WqT], axis=1)).astype(bf),
        "bqka": np.ascontiguousarray(np.concatenate([bq_, bk_, bq_, bk_], axis=0)),
        "bqkb": np.ascontiguousarray(np.concatenate([bk_, bq_, bk_, bq_], axis=0)),
        "wvT": np.ascontiguousarray((g * np.asarray(Wv, np.float32)).T).astype(bf),
        "ones": np.ones((128, 128), bf),
    }
    xres = x + g * np.asarray(bv, np.float32)[None, :, None]  # (B, C, L)
    maps = []
    for b in range(B):
        m = dict(shared, x=np.ascontiguousarray(xres[b]).astype(bf))
        xbb = np.asarray(x[b]).astype(bf)
        for q in range(4):
            m[f"xb{q}"] = np.ascontiguousarray(xbb[:, q * 512:(q + 1) * 512])
        maps.append(m)
    return maps


def kernel(x, Wq, bq, Wk, bk, Wv, bv, gamma):
    from concourse.bass_utils import run_bass_kernel_spmd

    nc = get_nc()
    in_maps = make_in_maps(x, Wq, bq, Wk, bk, Wv, bv, gamma)
    res = run_bass_kernel_spmd(nc, in_maps, list(range(NCORES)))
    return np.stack([res.results[b]["y"] for b in range(B)], axis=0)


# revision 15
# speedup vs baseline: 1.0161x; 1.0153x over previous
"""Trainium2 Bass kernel for nn_AttentionModule (B=8, C=256, L=2048, D=32).

Per-batch computation (data-parallel: one batch per NeuronCore, 8 cores):
    qT = Wq @ x + bq            # (D, L)
    kT = Wk @ x + bk            # (D, L)
    vT = x.T @ (g*Wv).T         # (L, C)   -- gamma folded into Wv host-side
    ST = kT.T @ qT              # (L_j, L_i) = S[i,j] transposed
    E  = exp(ST)                # no max-subtraction: max|S| ~ 46, exp fits
    Z  = sum_j E[j, i]
    y  = U * (1/Z) + x'         # U = vT.T @ E;  x' = x + g*bv (host-folded)

Restructured v2: i processed in 2 halves of 1024 columns; inside a half the
16 j-blocks stream through a 3-stage pipeline (S matmul -> exp -> {U matmul,
Z accumulate}) with wide [128,1024] tiles so per-instruction overheads
amortize.  S uses 32-row PE packing with the row-group pair alternating
per j-block so next block's LDWEIGHTS overlaps in-flight matmuls.  Z is
partition-reduced with an all-ones 128x128 stationary, which lands the
row sums already broadcast across all partitions (no transpose / no
broadcast step); 1/Z via reciprocal_approx_fast.  gamma and the v/q-k
biases are folded into the weights / residual host-side.
"""

import numpy as np

B, C, L, D = 8, 256, 2048, 32
H = 1024  # i-half width
NCORES = 8

_cache = {}


def _build_nc():
    from contextlib import ExitStack

    import concourse.bacc as bacc
    import concourse.tile as tile
    from concourse import mybir

    f32 = mybir.dt.float32
    bf16 = mybir.dt.bfloat16
    EXP = mybir.ActivationFunctionType.Exp
    IDENT = mybir.ActivationFunctionType.Identity

    nc = bacc.Bacc("TRN2", target_bir_lowering=False, debug=False)

    x_d = nc.dram_tensor("x", [C, L], bf16, kind="ExternalInput")
    xbq_d = [nc.dram_tensor(f"xb{q}", [C, 512], bf16, kind="ExternalInput")
             for q in range(4)]
    wqka_d = nc.dram_tensor("wqka", [C, 128], bf16, kind="ExternalInput")
    wqkb_d = nc.dram_tensor("wqkb", [C, 128], bf16, kind="ExternalInput")
    wvT_d = nc.dram_tensor("wvT", [C, C], bf16, kind="ExternalInput")
    bqka_d = nc.dram_tensor("bqka", [128, 1], f32, kind="ExternalInput")
    bqkb_d = nc.dram_tensor("bqkb", [128, 1], f32, kind="ExternalInput")
    ones_d = nc.dram_tensor("ones", [128, 128], bf16, kind="ExternalInput")
    y_d = nc.dram_tensor("y", [C, L], f32, kind="ExternalOutput")

    x_ap = x_d.ap()
    y_ap = y_d.ap()

    with tile.TileContext(nc) as tc, ExitStack() as ctx:
        singles = ctx.enter_context(tc.tile_pool(name="singles", bufs=1))
        big = ctx.enter_context(tc.tile_pool(name="big", bufs=1))
        ps = ctx.enter_context(tc.tile_pool(name="ps", bufs=2, space="PSUM"))
        up = ctx.enter_context(tc.tile_pool(name="up", bufs=1, space="PSUM"))
        epool = ctx.enter_context(tc.tile_pool(name="epool", bufs=4))
        ypool = ctx.enter_context(tc.tile_pool(name="ypool", bufs=4))
        zpool = ctx.enter_context(tc.tile_pool(name="zpool", bufs=1))
        rpool = ctx.enter_context(tc.tile_pool(name="rpool", bufs=2))

        # ---- load compute inputs (weights first, then xb; bf16 residual last) ----
        wqka_sb, wqkb_sb, wvT_sb = [], [], []
        for ct in range(2):
            ta = singles.tile([128, 128], bf16, tag=f"wqka{ct}")
            nc.sync.dma_start(out=ta[:], in_=wqka_d.ap()[ct * 128:(ct + 1) * 128, :])
            wqka_sb.append(ta)
            tb2 = singles.tile([128, 128], bf16, tag=f"wqkb{ct}")
            nc.sync.dma_start(out=tb2[:], in_=wqkb_d.ap()[ct * 128:(ct + 1) * 128, :])
            wqkb_sb.append(tb2)
            tv = singles.tile([128, C], bf16, tag=f"wv{ct}")
            nc.sync.dma_start(out=tv[:], in_=wvT_d.ap()[ct * 128:(ct + 1) * 128, :])
            wvT_sb.append(tv)
        bqka_sb = singles.tile([128, 1], f32, tag="bqka")
        nc.sync.dma_start(out=bqka_sb[:], in_=bqka_d.ap()[:, :])
        bqkb_sb = singles.tile([128, 1], f32, tag="bqkb")
        nc.sync.dma_start(out=bqkb_sb[:], in_=bqkb_d.ap()[:, :])
        ones_sb = singles.tile([128, 128], bf16, tag="ones")
        nc.sync.dma_start(out=ones_sb[:], in_=ones_d.ap()[:, :])
        xb_sb = []
        for ct in range(2):
            tb = big.tile([128, L], bf16, tag=f"xb{ct}")
            xb_sb.append(tb)
        for q in range(4):
            for ct in range(2):
                nc.sync.dma_start(
                    out=xb_sb[ct][:, q * 512:(q + 1) * 512],
                    in_=xbq_d[q].ap()[ct * 128:(ct + 1) * 128, :],
                )

        # ---- PE warm-up: tiny matmuls keep the HAM clock gate open while
        # input DMAs land, so the projections run at 2.4 GHz not 1.2.
        warmsrc = singles.tile([128, 64], bf16, tag="warmsrc")
        warm_e = singles.tile([1, 2], bf16, tag="warm_e")
        nc.gpsimd.memset(warmsrc[:], 1.0)
        warmps = ps.tile([1, 64], f32, tag="stp", name="warmps")
        # hoist the exp ACT_TABLE_LOAD off the critical path
        nc.scalar.activation(warm_e[:], warmsrc[0:1, 0:2], EXP)

        def warm_burst(src, n):
            for _ in range(n):
                nc.tensor.matmul(
                    warmps[0:1, :], lhsT=src, rhs=warmsrc[:, :],
                    start=True, stop=True,
                )
        warm_burst(warmsrc[:, 0:1], 40)
        warm_burst(wqka_sb[0][:, 0:1], 12)
        warm_burst(wvT_sb[0][:, 0:1], 12)

        # ---- q/k projection, strip-interleaved ----
        # proja strips = [q|k|q|k], projb strips = [k|q|k|q]: every 32-row
        # strip g has q in one tile and k in the other, so the packed score
        # matmuls need no replication DMAs at all.
        proja = big.tile([128, L], bf16, tag="proja")
        projb = big.tile([128, L], bf16, tag="projb")
        for p in range(2):
            for tile_w, tile_b, dst in ((wqka_sb, bqka_sb, proja),
                                        (wqkb_sb, bqkb_sb, projb)):
                qkp = up.tile([128, H], f32, tag=f"u{0 if dst is proja else 1}",
                              name="qkp")
                for hi in range(2):
                    it = 2 * p + hi
                    for ct in range(2):
                        nc.tensor.matmul(
                            qkp[:, hi * 512:(hi + 1) * 512],
                            lhsT=tile_w[ct][:],
                            rhs=xb_sb[ct][:, it * 512:(it + 1) * 512],
                            start=(ct == 0),
                            stop=(ct == 1),
                        )
                nc.scalar.activation(
                    dst[:, p * H:(p + 1) * H], qkp[:, :], IDENT, bias=tile_b[:],
                )

        # ---- v projection + attention ----
        vT_sb = big.tile([128, 16 * C], bf16, tag="vT")

        def emit_vgrp(grp):
            vp = up.tile([128, H], f32, tag=f"u{grp % 2}", name="vp")
            for q4 in range(4):
                lb = 4 * grp + q4
                for ct in range(2):
                    nc.tensor.matmul(
                        vp[:, q4 * C:(q4 + 1) * C],
                        lhsT=xb_sb[ct][:, lb * 128:(lb + 1) * 128],
                        rhs=wvT_sb[ct][:],
                        start=(ct == 0),
                        stop=(ct == 1),
                    )
            nc.vector.tensor_copy(vT_sb[:, grp * H:(grp + 1) * H], vp[:, :])

        # S for block jb+1 is emitted BEFORE U for block jb so the in-order
        # PE queue computes next scores while ACT runs exp on the current
        # block (software pipelining).
        def emit_S(h, jb):
            gp = 2 * (jb % 2)  # alternate row-group pairs {0,1} / {2,3}
            stp = ps.tile([128, H], f32, tag="stp", name="stp")
            for c2 in range(2):
                g = gp + c2
                ktile = projb if g % 2 == 0 else proja
                qtile = proja if g % 2 == 0 else projb
                nc.tensor.matmul(
                    stp[:, c2 * 512:(c2 + 1) * 512],
                    lhsT=ktile[32 * g:32 * (g + 1), jb * 128:(jb + 1) * 128],
                    rhs=qtile[32 * g:32 * (g + 1), h * H + c2 * 512:h * H + (c2 + 1) * 512],
                    start=True,
                    stop=True,
                    tile_position=(32 * g, 0),
                )
            return stp

        emit_vgrp(0)
        stp_q = [emit_S(0, 0)]
        emit_vgrp(1)
        stp_q.append(emit_S(0, 1))
        emit_vgrp(2)
        emit_vgrp(3)

        # fp32 residual x' = x + g*bv (folded host-side)
        x_sb = []
        for ct in range(2):
            t = big.tile([128, L], bf16, tag=f"x{ct}")
            nc.sync.dma_start(out=t[:], in_=x_ap[ct * 128:(ct + 1) * 128, :])
            x_sb.append(t)

        # ---- attention, two i-halves of 1024 ----
        for h in range(2):
            zA = zpool.tile([128, H], bf16, tag="zA")
            zB = zpool.tile([128, H], bf16, tag="zB")
            u_t = [up.tile([128, H], f32, tag=f"u{ct}", name=f"u{ct}") for ct in range(2)]
            zp = None
            for jb in range(16):
                stp = stp_q.pop(0)
                e = epool.tile([128, H], bf16, tag="e")
                nc.scalar.activation(e[:], stp[:], EXP)
                nxt = jb + 2
                if nxt < 16:
                    stp_q.append(emit_S(h, nxt))
                elif h == 0:
                    stp_q.append(emit_S(1, nxt - 16))

                def emit_U():
                    for ct in range(2):
                        for c2 in range(2):
                            nc.tensor.matmul(
                                u_t[ct][:, c2 * 512:(c2 + 1) * 512],
                                lhsT=vT_sb[:, jb * C + ct * 128:jb * C + (ct + 1) * 128],
                                rhs=e[:, c2 * 512:(c2 + 1) * 512],
                                start=(jb == 0),
                                stop=(jb == 15),
                            )

                def emit_z15():
                    for c2 in range(2):
                        sl = slice(c2 * 512, (c2 + 1) * 512)
                        nc.tensor.matmul(zp[:, sl], lhsT=ones_sb[:], rhs=e[:, sl],
                                         start=False, stop=True)

                # Z accumulation: blocks 0-13 into two parity accumulators on
                # DVE; blocks 14/15 go straight into the Z-reduce matmul group
                # so the tail doesn't wait on the last two DVE adds.  On the
                # final half the block-15 Z matmuls go ahead of U so the
                # reciprocal overlaps the last U matmuls.
                if jb < 2:
                    zt = zA if jb % 2 == 0 else zB
                    emit_U()
                    nc.vector.tensor_copy(zt[:], e[:])
                elif jb < 13:
                    zt = zA if jb % 2 == 0 else zB
                    emit_U()
                    nc.vector.tensor_add(zt[:], zt[:], e[:])
                elif jb == 13:
                    emit_U()
                    nc.vector.tensor_add(zB[:], zB[:], e[:])
                    # Z partition-reduce; ones stationary broadcasts the sums.
                    # zA finished at block 12, reduce it now to drain the tail.
                    zp = ps.tile([128, H], f32, tag="stp", name="zp")
                    for c2 in range(2):
                        sl = slice(c2 * 512, (c2 + 1) * 512)
                        nc.tensor.matmul(zp[:, sl], lhsT=ones_sb[:], rhs=zA[:, sl],
                                         start=True, stop=False)
                elif jb == 14:
                    emit_U()
                    for c2 in range(2):
                        sl = slice(c2 * 512, (c2 + 1) * 512)
                        nc.tensor.matmul(zp[:, sl], lhsT=ones_sb[:], rhs=zB[:, sl],
                                         start=False, stop=False)
                        nc.tensor.matmul(zp[:, sl], lhsT=ones_sb[:], rhs=e[:, sl],
                                         start=False, stop=False)
                elif h == 1:
                    emit_z15()
                    emit_U()
                else:
                    emit_U()
                    emit_z15()
            rb = rpool.tile([128, H], f32, tag="rb")
            nc.vector.reciprocal_approx_fast(out=rb[:], in_=zp[:])
            for ct in range(2):
                yt = ypool.tile([128, H], f32, tag="y")
                nc.vector.tensor_mul(yt[:], u_t[ct][:], rb[:])
                eng = nc.gpsimd if ct == 0 else nc.vector
                eng.tensor_add(yt[:], yt[:], x_sb[ct][:, h * H:(h + 1) * H])
                nc.sync.dma_start(
                    out=y_ap[ct * 128:(ct + 1) * 128, h * H:(h + 1) * H], in_=yt[:]
                )

    nc.compile()
    return nc


def get_nc():
    if "nc" not in _cache:
        _cache["nc"] = _build_nc()
    return _cache["nc"]


def make_in_maps(x, Wq, bq, Wk, bk, Wv, bv, gamma):
    import ml_dtypes

    bf = ml_dtypes.bfloat16
    x = np.asarray(x, dtype=np.float32)
    g = float(np.asarray(gamma, np.float32).reshape(-1)[0])
    WqT = np.asarray(Wq, np.float32).T
    WkT = np.asarray(Wk, np.float32).T
    bq_ = np.asarray(bq, np.float32).reshape(D, 1)
    bk_ = np.asarray(bk, np.float32).reshape(D, 1)
    shared = {
        "wqka": np.ascontiguousarray(
            np.concatenate([WqT, WkT, WqT, WkT], axis=1)).astype(bf),
        "wqkb": np.ascontiguousarray(
            np.concatenate([WkT, WqT, WkT, WqT], axis=1)).astype(bf),
        "bqka": np.ascontiguousarray(np.concatenate([bq_, bk_, bq_, bk_], axis=0)),
        "bqkb": np.ascontiguousarray(np.concatenate([bk_, bq_, bk_, bq_], axis=0)),
        "wvT": np.ascontiguousarray((g * np.asarray(Wv, np.float32)).T).astype(bf),
        "ones": np.ones((128, 128), bf),
    }
    xres = x + g * np.asarray(bv, np.float32)[None, :, None]  # (B, C, L)
    maps = []
    for b in range(B):
        m = dict(shared, x=np.ascontiguousarray(xres[b]).astype(bf))
        xbb = np.asarray(x[b]).astype(bf)
        for q in range(4):
            m[f"xb{q}"] = np.ascontiguousarray(xbb[:, q * 512:(q + 1) * 512])
        maps.append(m)
    return maps


def kernel(x, Wq, bq, Wk, bk, Wv, bv, gamma):
    from concourse.bass_utils import run_bass_kernel_spmd

    nc = get_nc()
    in_maps = make_in_maps(x, Wq, bq, Wk, bk, Wv, bv, gamma)
    res = run_bass_kernel_spmd(nc, in_maps, list(range(NCORES)))
    return np.stack([res.results[b]["y"] for b in range(B)], axis=0)


# revision 16
# speedup vs baseline: 1.0731x; 1.0560x over previous
"""Trainium2 Bass kernel for nn_AttentionModule (B=8, C=256, L=2048, D=32).

Per-batch computation (data-parallel: one batch per NeuronCore, 8 cores):
    qT = Wq @ x + bq            # (D, L)
    kT = Wk @ x + bk            # (D, L)
    vT = x.T @ (g*Wv).T         # (L, C)   -- gamma folded into Wv host-side
    ST = kT.T @ qT              # (L_j, L_i) = S[i,j] transposed
    E  = exp(ST)                # no max-subtraction: max|S| ~ 46, exp fits
    Z  = sum_j E[j, i]
    y  = U * (1/Z) + x'         # U = vT.T @ E;  x' = x + g*bv (host-folded)

Restructured v2: i processed in 2 halves of 1024 columns; inside a half the
16 j-blocks stream through a 3-stage pipeline (S matmul -> exp -> {U matmul,
Z accumulate}) with wide [128,1024] tiles so per-instruction overheads
amortize.  S uses 32-row PE packing with the row-group pair alternating
per j-block so next block's LDWEIGHTS overlaps in-flight matmuls.  Z is
partition-reduced with an all-ones 128x128 stationary, which lands the
row sums already broadcast across all partitions (no transpose / no
broadcast step); 1/Z via reciprocal_approx_fast.  gamma and the v/q-k
biases are folded into the weights / residual host-side.
"""

import numpy as np

B, C, L, D = 8, 256, 2048, 32
H = 1024  # i-half width
NCORES = 8

_cache = {}


def _build_nc():
    from contextlib import ExitStack

    import concourse.bacc as bacc
    import concourse.tile as tile
    from concourse import mybir

    f32 = mybir.dt.float32
    bf16 = mybir.dt.bfloat16
    EXP = mybir.ActivationFunctionType.Exp
    IDENT = mybir.ActivationFunctionType.Identity

    nc = bacc.Bacc("TRN2", target_bir_lowering=False, debug=False)

    x_d = nc.dram_tensor("x", [C, L], bf16, kind="ExternalInput")
    xbq_d = [nc.dram_tensor(f"xb{q}", [C, 512], bf16, kind="ExternalInput")
             for q in range(4)]
    wqka_d = nc.dram_tensor("wqka", [C, 128], bf16, kind="ExternalInput")
    p32_d = nc.dram_tensor("p32", [128, 128], bf16, kind="ExternalInput")
    wvT_d = nc.dram_tensor("wvT", [C, C], bf16, kind="ExternalInput")
    bqka_d = nc.dram_tensor("bqka", [128, 1], f32, kind="ExternalInput")
    ones_d = nc.dram_tensor("ones", [128, 128], bf16, kind="ExternalInput")
    y_d = nc.dram_tensor("y", [C, L], f32, kind="ExternalOutput")

    x_ap = x_d.ap()
    y_ap = y_d.ap()

    with tile.TileContext(nc) as tc, ExitStack() as ctx:
        singles = ctx.enter_context(tc.tile_pool(name="singles", bufs=1))
        big = ctx.enter_context(tc.tile_pool(name="big", bufs=1))
        ps = ctx.enter_context(tc.tile_pool(name="ps", bufs=2, space="PSUM"))
        up = ctx.enter_context(tc.tile_pool(name="up", bufs=1, space="PSUM"))
        epool = ctx.enter_context(tc.tile_pool(name="epool", bufs=4))
        ypool = ctx.enter_context(tc.tile_pool(name="ypool", bufs=4))
        zpool = ctx.enter_context(tc.tile_pool(name="zpool", bufs=1))
        rpool = ctx.enter_context(tc.tile_pool(name="rpool", bufs=2))

        # ---- load compute inputs (weights first, then xb; bf16 residual last) ----
        wqka_sb, wvT_sb = [], []
        for ct in range(2):
            ta = singles.tile([128, 128], bf16, tag=f"wqka{ct}")
            nc.sync.dma_start(out=ta[:], in_=wqka_d.ap()[ct * 128:(ct + 1) * 128, :])
            wqka_sb.append(ta)
            tv = singles.tile([128, C], bf16, tag=f"wv{ct}")
            nc.sync.dma_start(out=tv[:], in_=wvT_d.ap()[ct * 128:(ct + 1) * 128, :])
            wvT_sb.append(tv)
        bqka_sb = singles.tile([128, 1], f32, tag="bqka")
        nc.sync.dma_start(out=bqka_sb[:], in_=bqka_d.ap()[:, :])
        p32_sb = singles.tile([128, 128], bf16, tag="p32")
        nc.sync.dma_start(out=p32_sb[:], in_=p32_d.ap()[:, :])
        ones_sb = singles.tile([128, 128], bf16, tag="ones")
        nc.sync.dma_start(out=ones_sb[:], in_=ones_d.ap()[:, :])
        xb_sb = []
        for ct in range(2):
            tb = big.tile([128, L], bf16, tag=f"xb{ct}")
            xb_sb.append(tb)
        for q in range(4):
            for ct in range(2):
                nc.sync.dma_start(
                    out=xb_sb[ct][:, q * 512:(q + 1) * 512],
                    in_=xbq_d[q].ap()[ct * 128:(ct + 1) * 128, :],
                )

        # ---- PE warm-up: tiny matmuls keep the HAM clock gate open while
        # input DMAs land, so the projections run at 2.4 GHz not 1.2.
        warmsrc = singles.tile([128, 64], bf16, tag="warmsrc")
        warm_e = singles.tile([1, 2], bf16, tag="warm_e")
        nc.gpsimd.memset(warmsrc[:], 1.0)
        warmps = ps.tile([1, 64], f32, tag="stp", name="warmps")
        # hoist the exp ACT_TABLE_LOAD off the critical path
        nc.scalar.activation(warm_e[:], warmsrc[0:1, 0:2], EXP)

        def warm_burst(src, n):
            for _ in range(n):
                nc.tensor.matmul(
                    warmps[0:1, :], lhsT=src, rhs=warmsrc[:, :],
                    start=True, stop=True,
                )
        warm_burst(warmsrc[:, 0:1], 110)

        # ---- q/k projection, strip-interleaved ----
        # proja strips = [q|k|q|k]; projb = proja rotated down 32 partitions
        # via a permutation matmul = [k|q|k|q].  Every 32-row strip g then has
        # q in one tile and k in the other: the packed score matmuls need no
        # replication DMAs and only one projection pass.
        proja = big.tile([128, L], bf16, tag="proja")
        projb = big.tile([128, L], bf16, tag="projb")
        for p in range(2):
            qkp = up.tile([128, H], f32, tag=f"u{p}", name="qkp")
            for hi in range(2):
                it = 2 * p + hi
                for ct in range(2):
                    nc.tensor.matmul(
                        qkp[:, hi * 512:(hi + 1) * 512],
                        lhsT=wqka_sb[ct][:],
                        rhs=xb_sb[ct][:, it * 512:(it + 1) * 512],
                        start=(ct == 0),
                        stop=(ct == 1),
                    )
            nc.scalar.activation(
                proja[:, p * H:(p + 1) * H], qkp[:, :], IDENT, bias=bqka_sb[:],
            )
            shp = ps.tile([128, H], f32, tag="stp", name="shp")
            for c2 in range(2):
                nc.tensor.matmul(
                    shp[:, c2 * 512:(c2 + 1) * 512],
                    lhsT=p32_sb[:],
                    rhs=proja[:, p * H + c2 * 512:p * H + (c2 + 1) * 512],
                    start=True,
                    stop=True,
                )
            nc.vector.tensor_copy(projb[:, p * H:(p + 1) * H], shp[:, :])

        # ---- v projection + attention ----
        vT_sb = big.tile([128, 16 * C], bf16, tag="vT")

        def emit_vgrp(grp):
            vp = up.tile([128, H], f32, tag=f"u{grp % 2}", name="vp")
            for q4 in range(4):
                lb = 4 * grp + q4
                for ct in range(2):
                    nc.tensor.matmul(
                        vp[:, q4 * C:(q4 + 1) * C],
                        lhsT=xb_sb[ct][:, lb * 128:(lb + 1) * 128],
                        rhs=wvT_sb[ct][:],
                        start=(ct == 0),
                        stop=(ct == 1),
                    )
            nc.vector.tensor_copy(vT_sb[:, grp * H:(grp + 1) * H], vp[:, :])

        # S for block jb+1 is emitted BEFORE U for block jb so the in-order
        # PE queue computes next scores while ACT runs exp on the current
        # block (software pipelining).
        def emit_S(h, jb):
            gp = 2 * (jb % 2)  # alternate row-group pairs {0,1} / {2,3}
            stp = ps.tile([128, H], f32, tag="stp", name="stp")
            for c2 in range(2):
                g = gp + c2
                ktile = projb if g % 2 == 0 else proja
                qtile = proja if g % 2 == 0 else projb
                nc.tensor.matmul(
                    stp[:, c2 * 512:(c2 + 1) * 512],
                    lhsT=ktile[32 * g:32 * (g + 1), jb * 128:(jb + 1) * 128],
                    rhs=qtile[32 * g:32 * (g + 1), h * H + c2 * 512:h * H + (c2 + 1) * 512],
                    start=True,
                    stop=True,
                    tile_position=(32 * g, 0),
                )
            return stp

        emit_vgrp(0)
        stp_q = [emit_S(0, 0)]
        emit_vgrp(1)
        stp_q.append(emit_S(0, 1))
        emit_vgrp(2)
        emit_vgrp(3)

        # fp32 residual x' = x + g*bv (folded host-side)
        x_sb = []
        for ct in range(2):
            t = big.tile([128, L], bf16, tag=f"x{ct}")
            nc.sync.dma_start(out=t[:], in_=x_ap[ct * 128:(ct + 1) * 128, :])
            x_sb.append(t)

        # ---- attention, two i-halves of 1024 ----
        for h in range(2):
            zA = zpool.tile([128, H], bf16, tag="zA")
            zB = zpool.tile([128, H], bf16, tag="zB")
            u_t = [up.tile([128, H], f32, tag=f"u{ct}", name=f"u{ct}") for ct in range(2)]
            zp = None
            for jb in range(16):
                stp = stp_q.pop(0)
                e = epool.tile([128, H], bf16, tag="e")
                nc.scalar.activation(e[:], stp[:], EXP)
                nxt = jb + 2
                if nxt < 16:
                    stp_q.append(emit_S(h, nxt))
                elif h == 0:
                    stp_q.append(emit_S(1, nxt - 16))

                def emit_U():
                    for ct in range(2):
                        for c2 in range(2):
                            nc.tensor.matmul(
                                u_t[ct][:, c2 * 512:(c2 + 1) * 512],
                                lhsT=vT_sb[:, jb * C + ct * 128:jb * C + (ct + 1) * 128],
                                rhs=e[:, c2 * 512:(c2 + 1) * 512],
                                start=(jb == 0),
                                stop=(jb == 15),
                            )

                def emit_z15():
                    for c2 in range(2):
                        sl = slice(c2 * 512, (c2 + 1) * 512)
                        nc.tensor.matmul(zp[:, sl], lhsT=ones_sb[:], rhs=e[:, sl],
                                         start=False, stop=True)

                # Z accumulation: blocks 0-13 into two parity accumulators on
                # DVE; blocks 14/15 go straight into the Z-reduce matmul group
                # so the tail doesn't wait on the last two DVE adds.  On the
                # final half the block-15 Z matmuls go ahead of U so the
                # reciprocal overlaps the last U matmuls.
                if jb < 2:
                    zt = zA if jb % 2 == 0 else zB
                    emit_U()
                    nc.vector.tensor_copy(zt[:], e[:])
                elif jb < 13:
                    zt = zA if jb % 2 == 0 else zB
                    emit_U()
                    nc.vector.tensor_add(zt[:], zt[:], e[:])
                elif jb == 13:
                    emit_U()
                    nc.vector.tensor_add(zB[:], zB[:], e[:])
                    # Z partition-reduce; ones stationary broadcasts the sums.
                    # zA finished at block 12, reduce it now to drain the tail.
                    zp = ps.tile([128, H], f32, tag="stp", name="zp")
                    for c2 in range(2):
                        sl = slice(c2 * 512, (c2 + 1) * 512)
                        nc.tensor.matmul(zp[:, sl], lhsT=ones_sb[:], rhs=zA[:, sl],
                                         start=True, stop=False)
                elif jb == 14:
                    emit_U()
                    for c2 in range(2):
                        sl = slice(c2 * 512, (c2 + 1) * 512)
                        nc.tensor.matmul(zp[:, sl], lhsT=ones_sb[:], rhs=zB[:, sl],
                                         start=False, stop=False)
                        nc.tensor.matmul(zp[:, sl], lhsT=ones_sb[:], rhs=e[:, sl],
                                         start=False, stop=False)
                elif h == 1:
                    emit_z15()
                    emit_U()
                else:
                    emit_U()
                    emit_z15()
            rb = rpool.tile([128, H], f32, tag="rb")
            nc.vector.reciprocal_approx_fast(out=rb[:], in_=zp[:])
            for ct in range(2):
                yt = ypool.tile([128, H], f32, tag="y")
                nc.vector.tensor_mul(yt[:], u_t[ct][:], rb[:])
                eng = nc.gpsimd if ct == 0 else nc.vector
                eng.tensor_add(yt[:], yt[:], x_sb[ct][:, h * H:(h + 1) * H])
                nc.sync.dma_start(
                    out=y_ap[ct * 128:(ct + 1) * 128, h * H:(h + 1) * H], in_=yt[:]
                )

    nc.compile()
    return nc


def get_nc():
    if "nc" not in _cache:
        _cache["nc"] = _build_nc()
    return _cache["nc"]


def make_in_maps(x, Wq, bq, Wk, bk, Wv, bv, gamma):
    import ml_dtypes

    bf = ml_dtypes.bfloat16
    x = np.asarray(x, dtype=np.float32)
    g = float(np.asarray(gamma, np.float32).reshape(-1)[0])
    WqT = np.asarray(Wq, np.float32).T
    WkT = np.asarray(Wk, np.float32).T
    bq_ = np.asarray(bq, np.float32).reshape(D, 1)
    bk_ = np.asarray(bk, np.float32).reshape(D, 1)
    p32 = np.zeros((128, 128), np.float32)
    for r in range(128):
        p32[(r + 32) % 128, r] = 1.0
    shared = {
        "wqka": np.ascontiguousarray(
            np.concatenate([WqT, WkT, WqT, WkT], axis=1)).astype(bf),
        "bqka": np.ascontiguousarray(np.concatenate([bq_, bk_, bq_, bk_], axis=0)),
        "p32": p32.astype(bf),
        "wvT": np.ascontiguousarray((g * np.asarray(Wv, np.float32)).T).astype(bf),
        "ones": np.ones((128, 128), bf),
    }
    xres = x + g * np.asarray(bv, np.float32)[None, :, None]  # (B, C, L)
    maps = []
    for b in range(B):
        m = dict(shared, x=np.ascontiguousarray(xres[b]).astype(bf))
        xbb = np.asarray(x[b]).astype(bf)
        for q in range(4):
            m[f"xb{q}"] = np.ascontiguousarray(xbb[:, q * 512:(q + 1) * 512])
        maps.append(m)
    return maps


def kernel(x, Wq, bq, Wk, bk, Wv, bv, gamma):
    from concourse.bass_utils import run_bass_kernel_spmd

    nc = get_nc()
    in_maps = make_in_maps(x, Wq, bq, Wk, bk, Wv, bv, gamma)
    res = run_bass_kernel_spmd(nc, in_maps, list(range(NCORES)))
    return np.stack([res.results[b]["y"] for b in range(B)], axis=0)
